# revision 13
# baseline (speedup 1.0000x reference)
"""Trainium2 Bass kernel for nn_CustomLoss (BCE + binary-KL loss).

reference math (per element pair s=logits[:, :38], r=logits[:, 38:], y=labels):
    loss_sum = 1.5*S_sp_s - 0.5*S_sp_r - S_sy - 0.5*S_qs + 0.5*S_qr
    q = sigmoid(r),  sp(x) = softplus(x) = -ln(sigmoid(-x))

Device strategy (pure data parallel, batch sharded across 8 cores), v3:
  * host casts inputs to bf16: lg=[s|r] (152B/row) + y (76B/row); per-core
    DMA drops 29.9MB -> 14.9MB and the matmul eats the DMA'd bf16 directly.
  * ACT (sigmoid table): SSR = sigmoid(-[s|r]) - ONE contiguous op per tile.
  * DVE: q = 1 - SR at 4x; y copy into [y|q] at 4x; pairwise product folds
    of SSR across k-groups (3 levels, one op per level covers both sides).
  * ACT (ln table, once at end): ln(folded) with accum_out ->
    per-partition sums of ln sigmoid(-x) = -sp(x) for the s and r halves.
  * TensorE: one matmul per 128-row group, stationary [y|q], moving [s|r],
    PSUM diagonals give S_sy, S_qs, S_qr.
  * Host combines the tiny per-core outputs in float64.
"""

import numpy as np

N_CLASSES = 38
B_FULL = 524288
N_CORES = 8
ROWS_PER_CORE = B_FULL // N_CORES  # 65536
P = 128

K_GROUPS = 64        # 128-row groups per big tile
FOLD_LVL = 5         # fold depth: ln work = 76/2^FOLD_LVL per row
NP_PSUM = 2          # parallel psum accumulators
LY_BUFS = 5          # input-tile ring depth (DMA prefetch slack)

_CACHE = {}


def build_program(rows=ROWS_PER_CORE, K=K_GROUPS, np_psum=NP_PSUM):
    """Build the per-core Bass program (SPMD: same program on all cores)."""
    import concourse.bacc as bacc
    import concourse.bass as bass
    import concourse.mybir as mybir
    from concourse.tile import TileContext

    f32 = mybir.dt.float32
    bf16 = mybir.dt.bfloat16
    AF = mybir.ActivationFunctionType

    C = N_CLASSES          # 38
    C2 = 2 * C             # 76
    assert rows % (P * (1 << FOLD_LVL)) == 0
    G_TOT = rows // P      # 128-row groups per core
    NP = np_psum

    # tile schedule in units of K-groups: small tiles at the head (the DMA
    # clocks ramp up over the first ~10us; tiny sigmoid chunks keep ACT fed)
    # and at the tail (short compute chain after the final DMA byte lands)
    if G_TOT >= 4 * K:
        KE = max(K // 4, 1 << FOLD_LVL)
        mid = G_TOT - 2 * KE - (K - KE)
        bts = [KE, K - KE] + [K] * (mid // K)
        rem = G_TOT - sum(bts) - 2 * KE
        if rem:
            bts.append(rem)
        bts += [KE, KE]
    else:
        assert G_TOT % K == 0
        bts = [K] * (G_TOT // K)
    assert sum(bts) == G_TOT and all(b % (1 << FOLD_LVL) == 0 for b in bts)

    nc = bacc.Bacc(
        "TRN2", target_bir_lowering=False, debug=False, num_devices=N_CORES
    )
    lgd = nc.declare_dram_parameter("lg", [rows, C2], bf16, isOutput=False)
    yd = nc.declare_dram_parameter("y", [rows, C], bf16, isOutput=False)
    mm_out = nc.declare_dram_parameter("mm_out", [C2, C2 * NP], f32, isOutput=True)
    acc_out = nc.declare_dram_parameter("acc_out", [P, 2], f32, isOutput=True)

    # partition-major layout: partition p owns a contiguous block of rows
    lgf = lgd[:].rearrange("(p g) m -> p (g m)", p=P)
    yf = yd[:].rearrange("(p g) m -> p (g m)", p=P)

    with TileContext(nc) as tc:
        with (
            tc.tile_pool(name="work", bufs=2) as work,
            tc.tile_pool(name="persist", bufs=1) as persist,
            tc.tile_pool(name="psum", bufs=1, space="PSUM") as psump,
        ):
            OUT_ACC = persist.tile([P, 2], f32)
            nfold = (G_TOT >> FOLD_LVL) * C2
            FSR = persist.tile([P, nfold], bf16)  # folded sigmoid products
            psums = [
                psump.tile([C2, C2], f32, name=f"ps{i}", tag=f"ps{i}")
                for i in range(NP)
            ]

            row0 = 0  # starting 128-row group index of this tile
            for bt, Kb in enumerate(bts):
                LG = work.tile([P, Kb * C2], bf16, name="LG", bufs=LY_BUFS)
                Y = work.tile([P, Kb * C], bf16, name="Y", bufs=LY_BUFS)
                nc.gpsimd.dma_start(
                    out=LG[:], in_=lgf[:, row0 * C2 : (row0 + Kb) * C2]
                )
                nc.gpsimd.dma_start(
                    out=Y[:], in_=yf[:, row0 * C : (row0 + Kb) * C]
                )
                LG3 = LG.rearrange("p (k m) -> p k m", m=C2)
                Y3 = Y.rearrange("p (k m) -> p k m", m=C)

                # sigmoid(-[s|r]): one fully-contiguous ACT op per tile
                SSR = work.tile([P, Kb * C2], bf16, name="SSR")
                nc.scalar.activation(SSR[:], LG[:], AF.Sigmoid, scale=-1.0)
                SSR3 = SSR.rearrange("p (k m) -> p k m", m=C2)

                # stationary operand [y | q] in bf16
                YQ = work.tile([P, Kb * C2], bf16, name="YQ")
                YQ3 = YQ.rearrange("p (k m) -> p k m", m=C2)
                nc.vector.tensor_copy(YQ3[:, :, 0:C], Y3)
                # q = 1 - sigmoid(-r)  == sigmoid(r)
                nc.vector.tensor_scalar(
                    YQ3[:, :, C:C2], SSR3[:, :, C:C2], 1.0, -1.0,
                    op0=mybir.AluOpType.subtract, op1=mybir.AluOpType.mult,
                )

                # product folds across adjacent k-groups; one op per level
                # covers both halves (s cols 0:38, r cols 38:76 per block)
                cur, kk = SSR, Kb
                for lvl in range(FOLD_LVL):
                    kk //= 2
                    last = lvl == FOLD_LVL - 1
                    dst = FSR if last else work.tile(
                        [P, kk * C2], bf16, name=f"Pf{lvl}", tag=f"Pf{lvl}"
                    )
                    c4 = cur.rearrange(
                        "p (k2 two m) -> p k2 two m", two=2, m=C2
                    )
                    if last:
                        out = dst.rearrange("p (g m) -> p g m", m=C2)[
                            :, (row0 >> FOLD_LVL) : (row0 + Kb) >> FOLD_LVL
                        ]
                    else:
                        out = dst.rearrange("p (k m) -> p k m", m=C2)
                    nc.vector.tensor_mul(out, c4[:, :, 0], c4[:, :, 1])
                    cur = dst

                # matmuls: psum += [y|q]^T @ [s|r] per group
                for k in range(Kb):
                    g = row0 + k
                    nc.tensor.matmul(
                        psums[g % NP][:],
                        YQ3[:, k],
                        LG3[:, k],
                        start=(g < NP),
                        stop=(g >= G_TOT - NP),
                    )
                row0 += Kb

            # single table swap to natural_log, then two accumulating lns:
            # accum = sum ln sigmoid(-x) = -sum softplus(x) per partition
            JUNK = persist.tile([P, nfold // 2], bf16)
            FSR3 = FSR.rearrange("p (g m) -> p g m", m=C2)
            J3 = JUNK.rearrange("p (g m) -> p g m", m=C)
            nc.scalar.activation(
                J3, FSR3[:, :, 0:C], AF.Ln, accum_out=OUT_ACC[:, 0:1]
            )
            nc.scalar.activation(
                J3, FSR3[:, :, C:C2], AF.Ln, accum_out=OUT_ACC[:, 1:2]
            )

            OUT_MM = persist.tile([C2, C2 * NP], f32)
            for i in range(NP):
                nc.vector.tensor_copy(OUT_MM[:, i * C2 : (i + 1) * C2], psums[i][:])
            nc.scalar.dma_start(out=mm_out[:], in_=OUT_MM[:])
            nc.scalar.dma_start(out=acc_out[:], in_=OUT_ACC[:])

    # Restrict the activation-table universe so Sigmoid resolves only in
    # sigmoid_and_others and Ln only in natural_log: exactly one table load
    # at the start and one swap before the final lns.
    from concourse.hw_specs import get_activation_tables

    all_tabs = get_activation_tables(nc.m.arch)
    patched = {}
    for name, fns in all_tabs.items():
        if name == "sigmoid_and_others":
            patched[name] = {f for f in fns if f.name != "Ln"}
        elif name == "natural_log":
            patched[name] = fns
        else:
            patched[name] = {
                f for f in fns if f.name not in ("Sigmoid", "Ln")
            }
    import concourse.bacc as bacc_mod

    orig = bacc_mod.get_activation_tables
    bacc_mod.get_activation_tables = lambda arch: patched
    try:
        nc.compile()
    finally:
        bacc_mod.get_activation_tables = orig
    return nc


def combine_core_outputs(mm, acc, np_psum=NP_PSUM):
    """Reduce one core's raw outputs to the weighted sum of loss elements."""
    C = N_CLASSES
    C2 = 2 * C
    mm = np.asarray(mm, dtype=np.float64)
    acc = np.asarray(acc, dtype=np.float64)
    M = np.zeros((C2, C2), dtype=np.float64)
    for i in range(np_psum):
        M += mm[:, i * C2 : (i + 1) * C2]
    A_s = acc[:, 0].sum()          # sum ln sigmoid(-s) = -sum sp(s)
    A_r = acc[:, 1].sum()          # sum ln sigmoid(-r) = -sum sp(r)
    d = np.arange(C)
    S_sy = M[d, d].sum()           # sum s*y
    S_qs = M[C + d, d].sum()       # sum q*s
    S_qr = M[C + d, C + d].sum()   # sum q*r
    return -1.5 * A_s + 0.5 * A_r - S_sy - 0.5 * S_qs + 0.5 * S_qr


def _pack_inputs(logits, labels):
    """Host-side shard prep: bf16 casts of [s|r] and y."""
    import ml_dtypes

    lg = logits.astype(ml_dtypes.bfloat16)
    y = labels.astype(ml_dtypes.bfloat16)
    return lg, y


def kernel(logits, labels, should_print=0):
    from concourse.bass_utils import run_bass_kernel_spmd

    logits = np.ascontiguousarray(np.asarray(logits, dtype=np.float32))
    labels = np.asarray(labels)
    B = logits.shape[0]
    rows = B // N_CORES

    lg, y = _pack_inputs(logits, labels)

    key = ("prog", rows, K_GROUPS, NP_PSUM)
    if key not in _CACHE:
        _CACHE[key] = build_program(rows, K_GROUPS, NP_PSUM)
    nc = _CACHE[key]

    in_maps = [
        {
            "lg": lg[c * rows : (c + 1) * rows],
            "y": y[c * rows : (c + 1) * rows],
        }
        for c in range(N_CORES)
    ]
    res = run_bass_kernel_spmd(nc, in_maps, list(range(N_CORES)))
    total = 0.0
    for r in res.results:
        total += combine_core_outputs(r["mm_out"], r["acc_out"])
    loss = total / (B * N_CLASSES)
    return np.float32(loss)
